# revision 26
# baseline (speedup 1.0000x reference)
"""Bahdanau (additive) attention kernel for Trainium2, 8 NeuronCores.

Full-input contract: kernel(**inputs) takes the unsharded numpy inputs and
returns the full [TQ, B, D] output. Internally shards (batch, query-half)
across 8 cores (B=4 x 2 halves of Tq), runs a Bass/Tile kernel per core via
run_bass_kernel_spmd, and reassembles.

Sparsity: masked value positions contribute exactly 0 to the softmax
(score + -1e9 -> exp underflows to 0), so the host gathers only the valid
value positions per batch (mask is input data), pads to a common TVE
(multiple of 8), and the device program is compiled for that TVE (cached).

Per-core program (b = batch, 128 local queries, TVE gathered positions):
  warmup matmuls flip the PE clock gate (HAM) during the input DMAs
  wqT[u,q] = sum_d W1[d,u] q[q,d]          (PE matmul, fp32 -> SBUF)
  wkT[u,v] = sum_d W2[d,u] v[v,d]          (PE matmul, fp32; stays in PSUM)
  g_q[u,v] = tanh(wkT[u,v] + wqT[u,q])     (ACT, per-partition bias = wqT[:,q])
  scores[q,v] = mka[v] + sum_u scale[u] g_q[u,v]
      (PE: K=1 bf16 mask matmul opens the PSUM accumulation, then per q a
       float32r matmul with sliding-window lhsT = sigma in column q, else 0)
  e = exp(scores) in bank slices, each with fused row-sum (ACT accum_out)
  ctx[q,d] = (1/ssum[q]) sum_v e[q,v] v[v,d]  (PE transpose + matmuls,
       chunk-pipelined with the exp slices; DVE scale on the output copy)
"""

import sys

if "/opt/trn_rl_repo" not in sys.path:
    sys.path.insert(0, "/opt/trn_rl_repo")

import numpy as np

TQ, TV, B, D, U = 256, 1024, 4, 128, 128
NCORES = 8
TQL = 128  # local queries per core (Tq=256 split in 2 per batch)
NEG_INF = -1e9

# Score-contraction matmul dtype: "f32r" (reduced fp32, ~1e-4 rel err) or
# "bf16" (~1.5e-3) or "f32" (exact but 4 cyc/row).
SCORE_DT = "f32r"

_CACHE = {}


def _bank_pieces(tve):
    """Split [0, tve) into PSUM-bank-aligned matmul slices (<=512 each)."""
    pieces = []
    a = 0
    while a < tve:
        n = min(512, tve - a)
        pieces.append((a, n))
        a += n
    return pieces


def _build_nc(tve):
    import concourse.bacc as bacc
    import concourse.mybir as mybir
    import concourse.tile as tile
    from contextlib import ExitStack

    f32 = mybir.dt.float32
    f32r = mybir.dt.float32r
    bf16 = mybir.dt.bfloat16
    AFT = mybir.ActivationFunctionType

    nc = bacc.Bacc("TRN2", target_bir_lowering=False, debug=False,
                   num_devices=NCORES)

    sdt = {"f32r": f32r, "bf16": bf16, "f32": f32}[SCORE_DT]

    NVC = -(-tve // 128)              # ctx chunks (last may be partial)
    pieces = _bank_pieces(tve)

    wpack = nc.dram_tensor("wpack", [D, 3 * 128], f32,
                           kind="ExternalInput").ap()
    vt = nc.dram_tensor("vt", [D, tve], f32, kind="ExternalInput").ap()
    vnp = nc.dram_tensor("vnp", [128, NVC * D], f32,
                         kind="ExternalInput").ap()
    sige = nc.dram_tensor("sige", [U, 2 * TQL - 1], sdt,
                          kind="ExternalInput").ap()
    mpack = nc.dram_tensor("mpack", [1, tve + TQL], bf16,
                           kind="ExternalInput").ap()
    ident = nc.dram_tensor("ident", [128, 128], f32, kind="ExternalInput").ap()
    out = nc.dram_tensor("out", [TQL, D], f32, kind="ExternalOutput").ap()

    with tile.TileContext(nc) as tc:
        with ExitStack() as ctx:
            consts = ctx.enter_context(tc.tile_pool(name="consts", bufs=1))
            gpool = ctx.enter_context(tc.tile_pool(name="g", bufs=2))
            smp = ctx.enter_context(tc.tile_pool(name="sm", bufs=1))
            etp = ctx.enter_context(tc.tile_pool(name="et", bufs=2))
            ps1 = ctx.enter_context(tc.tile_pool(name="ps1", bufs=1,
                                                 space="PSUM"))
            pst = ctx.enter_context(tc.tile_pool(name="pst", bufs=2,
                                                 space="PSUM"))

            wpack_sb = consts.tile([D, 3 * 128], f32, tag="wpack")
            w1_sb = wpack_sb[:, 0:128]
            qt_sb = wpack_sb[:, 128:256]
            w2_sb = wpack_sb[:, 256:384]
            vt_sb = consts.tile([D, tve], f32, tag="vt")
            vnp_sb = consts.tile([128, NVC * D], f32, tag="vnp")
            sig_sb = consts.tile([U, 2 * TQL - 1], sdt, tag="sig")
            mpack_sb = consts.tile([1, tve + TQL], bf16, tag="mpack")
            mka_sb = mpack_sb[:, 0:tve]
            ones_sb = mpack_sb[:, tve:tve + TQL]
            id_sb = consts.tile([128, 128], f32, tag="id")
            wqT_sb = consts.tile([U, TQL], f32, tag="wqT")

            # preload the exp/tanh ACT table set during the input DMAs
            warm_in = consts.tile([128, 1], f32, tag="warm_in")
            warm_out = consts.tile([128, 1], f32, tag="warm_out")
            nc.gpsimd.memset(warm_in[:], 0.0)
            nc.scalar.activation(warm_out[:], warm_in[:], AFT.Tanh)

            # flip the PE clock gate (HAM) warm with ~3.4us of dummy
            # matmuls while the input DMAs land + complete
            warm_mm = consts.tile([128, 512], bf16, tag="warm_mm")
            nc.gpsimd.memset(warm_mm[:], 0.0)
            warm_ps = pst.tile([128, 512], f32, tag="tp")
            for _ in range(6):
                nc.tensor.matmul(warm_ps[:], lhsT=warm_mm[:, 0:128],
                                 rhs=warm_mm[:], start=True, stop=True)

            # one serialized DMA queue, priority order: the 16 SDMA engines
            # are shared, so concurrent bulk DMAs would delay vt's completion
            # receipt (which gates wk -> first tanh)
            nc.sync.dma_start(vt_sb[:], vt[:])
            nc.sync.dma_start(wpack_sb[:], wpack[:])
            nc.scalar.dma_start(sig_sb[:], sige[:])
            nc.sync.dma_start(mpack_sb[:], mpack[:])
            nc.sync.dma_start(id_sb[:], ident[:])
            nc.sync.dma_start(vnp_sb[:], vnp[:])

            # wkT = W2.T @ vT  -> stays in PSUM (ACT reads PSUM cheaper)
            wk_ps = ps1.tile([U, tve], f32, tag="wk")
            for a, n in pieces:
                nc.tensor.matmul(wk_ps[:, a:a + n], lhsT=w2_sb[:],
                                 rhs=vt_sb[:, a:a + n])

            # wqT = W1.T @ qT  -> copy to SBUF (ACT bias source)
            wq_ps = ps1.tile([U, TQL], f32, tag="wq")
            nc.tensor.matmul(wq_ps[:], lhsT=w1_sb[:], rhs=qt_sb[:])
            nc.vector.tensor_copy(wqT_sb[:], wq_ps[:])

            scores_ps = ps1.tile([TQL, tve], f32, tag="scores")
            # pad/mask add opens+closes the accumulation-group bookkeeping:
            # scores[m, v] = mka[v]; later matmuls accumulate per-element.
            for a, n in pieces:
                nc.tensor.matmul(scores_ps[:, a:a + n],
                                 lhsT=ones_sb[:], rhs=mka_sb[:, a:a + n],
                                 start=True, stop=True)
            QPACK = 16
            for q0 in range(0, TQL, QPACK):
                g2 = gpool.tile([U, QPACK, tve], sdt, tag="g")
                for i in range(QPACK):
                    q = q0 + i
                    nc.scalar.activation(g2[:, i, :], wk_ps[:], AFT.Tanh,
                                         bias=wqT_sb[:, q:q + 1])
                    lw = sig_sb[:, TQL - 1 - q: 2 * TQL - 1 - q]
                    for a, n in pieces:
                        nc.tensor.matmul(scores_ps[:, a:a + n],
                                         lhsT=lw, rhs=g2[:, i, a:a + n],
                                         start=False, stop=False,
                                         skip_group_check=True)

            # exp in bank slices with fused row-sums; ctx chunks pipeline in
            exp_sb = smp.tile([TQL, tve], f32, tag="exp")
            ssums = smp.tile([TQL, len(pieces)], f32, tag="ssums")
            ssum = smp.tile([TQL, 1], f32, tag="ssum")
            rins = smp.tile([TQL, 1], f32, tag="rins")
            for j, (a, n) in enumerate(pieces):
                nc.scalar.activation(exp_sb[:, a:a + n], scores_ps[:, a:a + n],
                                     AFT.Exp, accum_out=ssums[:, j:j + 1])
            if len(pieces) > 1:
                nc.vector.reduce_sum(ssum[:], ssums[:],
                                     axis=mybir.AxisListType.X)
            else:
                nc.vector.tensor_copy(ssum[:], ssums[:])
            nc.vector.reciprocal(rins[:], ssum[:])

            # ctx = softmax @ v  (transpose exp chunks, accumulate matmuls)
            ctx_ps = ps1.tile([TQL, D], f32, tag="ctx")
            for k in range(NVC):
                n = min(128, tve - k * 128)
                tp = pst.tile([128, 128], f32, tag="tp")
                nc.tensor.transpose(tp[:n, :],
                                    exp_sb[:, k * 128:k * 128 + n], id_sb[:])
                et = etp.tile([128, 128], f32, tag="et")
                nc.vector.tensor_copy(et[:n, :], tp[:n, :])
                nc.tensor.matmul(ctx_ps[:], lhsT=et[:n, :],
                                 rhs=vnp_sb[:n, k * D:(k + 1) * D],
                                 start=(k == 0), stop=(k == NVC - 1))

            out_sb = smp.tile([TQL, D], f32, tag="out")
            nc.vector.tensor_scalar_mul(out_sb[:], ctx_ps[:], rins[:])
            nc.sync.dma_start(out[:], out_sb[:])

    nc.compile()
    return nc


def get_nc(tve=TV):
    key = ("nc", tve)
    if key not in _CACHE:
        _CACHE[key] = _build_nc(tve)
    return _CACHE[key]


def prep_in_maps(query, value, mask, W1, W2, scale):
    """Gather valid value positions per batch; returns (in_maps, tve)."""
    import ml_dtypes

    query = np.asarray(query, dtype=np.float32)
    value = np.asarray(value, dtype=np.float32)
    mask = np.asarray(mask)
    W1 = np.ascontiguousarray(np.asarray(W1, dtype=np.float32))
    W2 = np.ascontiguousarray(np.asarray(W2, dtype=np.float32))
    scale = np.asarray(scale, dtype=np.float32)

    idxs = [np.nonzero(mask[:, b])[0] for b in range(B)]
    nv_max = max(1, max(len(ix) for ix in idxs))
    tve = min(TV, -(-nv_max // 4) * 4)
    NVC = -(-tve // 128)

    bf16_np = np.dtype(ml_dtypes.bfloat16)
    sdt_np = bf16_np if SCORE_DT == "bf16" else np.float32
    sige = np.zeros((U, 2 * TQL - 1), sdt_np)
    sige[:, TQL - 1] = scale.astype(sdt_np)
    ident = np.eye(128, dtype=np.float32)
    ones1 = np.ones((1, TQL), bf16_np)

    in_maps = []
    for c in range(NCORES):
        b, q0 = c // 2, (c % 2) * TQL
        ix = idxs[b]
        nv = len(ix)
        vg = np.zeros((NVC * 128, D), np.float32)
        vg[:nv] = value[ix, b, :]
        mka = np.zeros((1, tve), bf16_np)
        mka[0, nv:] = NEG_INF
        wpack = np.concatenate(
            [W1, np.ascontiguousarray(query[q0:q0 + TQL, b, :].T), W2],
            axis=1)
        mpack = np.concatenate([mka, ones1], axis=1)
        in_maps.append({
            "wpack": np.ascontiguousarray(wpack),
            "vt": np.ascontiguousarray(vg[:tve].T),
            "vnp": np.ascontiguousarray(
                vg.reshape(NVC, 128, D).transpose(1, 0, 2)
                .reshape(128, NVC * D)),
            "sige": sige,
            "mpack": np.ascontiguousarray(mpack),
            "ident": ident,
        })
    return in_maps, tve


def run(query, value, mask, W1, W2, scale, trace=False):
    from concourse.bass_utils import run_bass_kernel_spmd

    in_maps, tve = prep_in_maps(query, value, mask, W1, W2, scale)
    nc = get_nc(tve)
    res = run_bass_kernel_spmd(nc, in_maps, list(range(NCORES)), trace=trace)
    out = np.empty((TQ, B, D), np.float32)
    for c in range(NCORES):
        b, q0 = c // 2, (c % 2) * TQL
        out[q0:q0 + TQL, b, :] = res.results[c]["out"]
    return out, res


def kernel(query, value, mask, W1, W2, scale):
    out, _ = run(query, value, mask, W1, W2, scale, trace=False)
    return out


# revision 27
# speedup vs baseline: 1.0138x; 1.0138x over previous
"""Bahdanau (additive) attention kernel for Trainium2, 8 NeuronCores.

Full-input contract: kernel(**inputs) takes the unsharded numpy inputs and
returns the full [TQ, B, D] output. Internally shards (batch, query-half)
across 8 cores (B=4 x 2 halves of Tq), runs a Bass/Tile kernel per core via
run_bass_kernel_spmd, and reassembles.

Sparsity: masked value positions contribute exactly 0 to the softmax
(score + -1e9 -> exp underflows to 0), so the host gathers only the valid
value positions per batch (mask is input data), pads to a common TVE
(multiple of 8), and the device program is compiled for that TVE (cached).

Per-core program (b = batch, 128 local queries, TVE gathered positions):
  warmup matmuls flip the PE clock gate (HAM) during the input DMAs
  wqT[u,q] = sum_d W1[d,u] q[q,d]          (PE matmul, fp32 -> SBUF)
  wkT[u,v] = sum_d W2[d,u] v[v,d]          (PE matmul, fp32; stays in PSUM)
  g_q[u,v] = tanh(wkT[u,v] + wqT[u,q])     (ACT, per-partition bias = wqT[:,q])
  scores[q,v] = mka[v] + sum_u scale[u] g_q[u,v]
      (PE: K=1 bf16 mask matmul opens the PSUM accumulation, then per q a
       float32r matmul with sliding-window lhsT = sigma in column q, else 0)
  e = exp(scores) in bank slices, each with fused row-sum (ACT accum_out)
  ctx[q,d] = (1/ssum[q]) sum_v e[q,v] v[v,d]  (PE transpose + matmuls,
       chunk-pipelined with the exp slices; DVE scale on the output copy)
"""

import sys

if "/opt/trn_rl_repo" not in sys.path:
    sys.path.insert(0, "/opt/trn_rl_repo")

import numpy as np

TQ, TV, B, D, U = 256, 1024, 4, 128, 128
NCORES = 8
TQL = 128  # local queries per core (Tq=256 split in 2 per batch)
NEG_INF = -1e9

# Score-contraction matmul dtype: "f32r" (reduced fp32, ~1e-4 rel err) or
# "bf16" (~1.5e-3) or "f32" (exact but 4 cyc/row).
SCORE_DT = "f32r"

_CACHE = {}


def _bank_pieces(tve):
    """Split [0, tve) into PSUM-bank-aligned matmul slices (<=512 each)."""
    pieces = []
    a = 0
    while a < tve:
        n = min(512, tve - a)
        pieces.append((a, n))
        a += n
    return pieces


def _build_nc(tve):
    import concourse.bacc as bacc
    import concourse.mybir as mybir
    import concourse.tile as tile
    from contextlib import ExitStack

    f32 = mybir.dt.float32
    f32r = mybir.dt.float32r
    bf16 = mybir.dt.bfloat16
    AFT = mybir.ActivationFunctionType

    nc = bacc.Bacc("TRN2", target_bir_lowering=False, debug=False,
                   num_devices=NCORES)

    sdt = {"f32r": f32r, "bf16": bf16, "f32": f32}[SCORE_DT]

    NVC = -(-tve // 128)              # ctx chunks (last may be partial)
    pieces = _bank_pieces(tve)

    wpack = nc.dram_tensor("wpack", [D, 3 * 128], f32,
                           kind="ExternalInput").ap()
    vt = nc.dram_tensor("vt", [D, tve], f32, kind="ExternalInput").ap()
    vnp = nc.dram_tensor("vnp", [128, NVC * D], f32,
                         kind="ExternalInput").ap()
    sige = nc.dram_tensor("sige", [U, 2 * TQL - 1], sdt,
                          kind="ExternalInput").ap()
    mpack = nc.dram_tensor("mpack", [1, tve + TQL], bf16,
                           kind="ExternalInput").ap()
    ident = nc.dram_tensor("ident", [128, 128], f32, kind="ExternalInput").ap()
    out = nc.dram_tensor("out", [TQL, D], f32, kind="ExternalOutput").ap()

    with tile.TileContext(nc) as tc:
        with ExitStack() as ctx:
            consts = ctx.enter_context(tc.tile_pool(name="consts", bufs=1))
            gpool = ctx.enter_context(tc.tile_pool(name="g", bufs=3))
            smp = ctx.enter_context(tc.tile_pool(name="sm", bufs=1))
            etp = ctx.enter_context(tc.tile_pool(name="et", bufs=2))
            ps1 = ctx.enter_context(tc.tile_pool(name="ps1", bufs=1,
                                                 space="PSUM"))
            pst = ctx.enter_context(tc.tile_pool(name="pst", bufs=2,
                                                 space="PSUM"))

            wpack_sb = consts.tile([D, 3 * 128], f32, tag="wpack")
            w1_sb = wpack_sb[:, 0:128]
            qt_sb = wpack_sb[:, 128:256]
            w2_sb = wpack_sb[:, 256:384]
            vt_sb = consts.tile([D, tve], f32, tag="vt")
            vnp_sb = consts.tile([128, NVC * D], f32, tag="vnp")
            sig_sb = consts.tile([U, 2 * TQL - 1], sdt, tag="sig")
            mpack_sb = consts.tile([1, tve + TQL], bf16, tag="mpack")
            mka_sb = mpack_sb[:, 0:tve]
            ones_sb = mpack_sb[:, tve:tve + TQL]
            id_sb = consts.tile([128, 128], f32, tag="id")
            wqT_sb = consts.tile([U, TQL], f32, tag="wqT")

            # preload the exp/tanh ACT table set during the input DMAs
            warm_in = consts.tile([128, 1], f32, tag="warm_in")
            warm_out = consts.tile([128, 1], f32, tag="warm_out")
            nc.gpsimd.memset(warm_in[:], 0.0)
            nc.scalar.activation(warm_out[:], warm_in[:], AFT.Tanh)

            # flip the PE clock gate (HAM) warm with ~3.4us of dummy
            # matmuls while the input DMAs land + complete
            warm_mm = consts.tile([128, 512], bf16, tag="warm_mm")
            nc.gpsimd.memset(warm_mm[:], 0.0)
            warm_ps = pst.tile([128, 512], f32, tag="tp")
            for _ in range(6):
                nc.tensor.matmul(warm_ps[:], lhsT=warm_mm[:, 0:128],
                                 rhs=warm_mm[:], start=True, stop=True)

            # one serialized DMA queue, priority order: the 16 SDMA engines
            # are shared, so concurrent bulk DMAs would delay vt's completion
            # receipt (which gates wk -> first tanh)
            nc.sync.dma_start(vt_sb[:], vt[:])
            nc.sync.dma_start(wpack_sb[:], wpack[:])
            nc.scalar.dma_start(sig_sb[:], sige[:])
            nc.sync.dma_start(mpack_sb[:], mpack[:])
            nc.sync.dma_start(id_sb[:], ident[:])
            nc.sync.dma_start(vnp_sb[:], vnp[:])

            # wkT = W2.T @ vT  -> stays in PSUM (ACT reads PSUM cheaper)
            wk_ps = ps1.tile([U, tve], f32, tag="wk")
            for a, n in pieces:
                nc.tensor.matmul(wk_ps[:, a:a + n], lhsT=w2_sb[:],
                                 rhs=vt_sb[:, a:a + n])

            # wqT = W1.T @ qT  -> copy to SBUF (ACT bias source)
            wq_ps = ps1.tile([U, TQL], f32, tag="wq")
            nc.tensor.matmul(wq_ps[:], lhsT=w1_sb[:], rhs=qt_sb[:])
            nc.vector.tensor_copy(wqT_sb[:], wq_ps[:])

            scores_ps = ps1.tile([TQL, tve], f32, tag="scores")
            # pad/mask add opens+closes the accumulation-group bookkeeping:
            # scores[m, v] = mka[v]; later matmuls accumulate per-element.
            for a, n in pieces:
                nc.tensor.matmul(scores_ps[:, a:a + n],
                                 lhsT=ones_sb[:], rhs=mka_sb[:, a:a + n],
                                 start=True, stop=True)
            QPACK = 16
            for q0 in range(0, TQL, QPACK):
                g2 = gpool.tile([U, QPACK, tve], sdt, tag="g")
                for i in range(QPACK):
                    q = q0 + i
                    nc.scalar.activation(g2[:, i, :], wk_ps[:], AFT.Tanh,
                                         bias=wqT_sb[:, q:q + 1])
                    lw = sig_sb[:, TQL - 1 - q: 2 * TQL - 1 - q]
                    for a, n in pieces:
                        nc.tensor.matmul(scores_ps[:, a:a + n],
                                         lhsT=lw, rhs=g2[:, i, a:a + n],
                                         start=False, stop=False,
                                         skip_group_check=True)

            # exp in bank slices with fused row-sums; ctx chunks pipeline in
            exp_sb = smp.tile([TQL, tve], f32, tag="exp")
            ssums = smp.tile([TQL, len(pieces)], f32, tag="ssums")
            ssum = smp.tile([TQL, 1], f32, tag="ssum")
            rins = smp.tile([TQL, 1], f32, tag="rins")
            for j, (a, n) in enumerate(pieces):
                nc.scalar.activation(exp_sb[:, a:a + n], scores_ps[:, a:a + n],
                                     AFT.Exp, accum_out=ssums[:, j:j + 1])
            if len(pieces) > 1:
                nc.vector.reduce_sum(ssum[:], ssums[:],
                                     axis=mybir.AxisListType.X)
            else:
                nc.vector.tensor_copy(ssum[:], ssums[:])
            nc.vector.reciprocal(rins[:], ssum[:])

            # ctx = softmax @ v  (transpose exp chunks, accumulate matmuls)
            ctx_ps = ps1.tile([TQL, D], f32, tag="ctx")
            for k in range(NVC):
                n = min(128, tve - k * 128)
                tp = pst.tile([128, 128], f32, tag="tp")
                nc.tensor.transpose(tp[:n, :],
                                    exp_sb[:, k * 128:k * 128 + n], id_sb[:])
                et = etp.tile([128, 128], f32, tag="et")
                nc.vector.tensor_copy(et[:n, :], tp[:n, :])
                nc.tensor.matmul(ctx_ps[:], lhsT=et[:n, :],
                                 rhs=vnp_sb[:n, k * D:(k + 1) * D],
                                 start=(k == 0), stop=(k == NVC - 1))

            out_sb = smp.tile([TQL, D], f32, tag="out")
            nc.vector.tensor_scalar_mul(out_sb[:], ctx_ps[:], rins[:])
            nc.sync.dma_start(out[:], out_sb[:])

    nc.compile()
    return nc


def get_nc(tve=TV):
    key = ("nc", tve)
    if key not in _CACHE:
        _CACHE[key] = _build_nc(tve)
    return _CACHE[key]


def prep_in_maps(query, value, mask, W1, W2, scale):
    """Gather valid value positions per batch; returns (in_maps, tve)."""
    import ml_dtypes

    query = np.asarray(query, dtype=np.float32)
    value = np.asarray(value, dtype=np.float32)
    mask = np.asarray(mask)
    W1 = np.ascontiguousarray(np.asarray(W1, dtype=np.float32))
    W2 = np.ascontiguousarray(np.asarray(W2, dtype=np.float32))
    scale = np.asarray(scale, dtype=np.float32)

    idxs = [np.nonzero(mask[:, b])[0] for b in range(B)]
    nv_max = max(1, max(len(ix) for ix in idxs))
    tve = min(TV, -(-nv_max // 4) * 4)
    NVC = -(-tve // 128)

    bf16_np = np.dtype(ml_dtypes.bfloat16)
    sdt_np = bf16_np if SCORE_DT == "bf16" else np.float32
    sige = np.zeros((U, 2 * TQL - 1), sdt_np)
    sige[:, TQL - 1] = scale.astype(sdt_np)
    ident = np.eye(128, dtype=np.float32)
    ones1 = np.ones((1, TQL), bf16_np)

    in_maps = []
    for c in range(NCORES):
        b, q0 = c // 2, (c % 2) * TQL
        ix = idxs[b]
        nv = len(ix)
        vg = np.zeros((NVC * 128, D), np.float32)
        vg[:nv] = value[ix, b, :]
        mka = np.zeros((1, tve), bf16_np)
        mka[0, nv:] = NEG_INF
        wpack = np.concatenate(
            [W1, np.ascontiguousarray(query[q0:q0 + TQL, b, :].T), W2],
            axis=1)
        mpack = np.concatenate([mka, ones1], axis=1)
        in_maps.append({
            "wpack": np.ascontiguousarray(wpack),
            "vt": np.ascontiguousarray(vg[:tve].T),
            "vnp": np.ascontiguousarray(
                vg.reshape(NVC, 128, D).transpose(1, 0, 2)
                .reshape(128, NVC * D)),
            "sige": sige,
            "mpack": np.ascontiguousarray(mpack),
            "ident": ident,
        })
    return in_maps, tve


def run(query, value, mask, W1, W2, scale, trace=False):
    from concourse.bass_utils import run_bass_kernel_spmd

    in_maps, tve = prep_in_maps(query, value, mask, W1, W2, scale)
    nc = get_nc(tve)
    res = run_bass_kernel_spmd(nc, in_maps, list(range(NCORES)), trace=trace)
    out = np.empty((TQ, B, D), np.float32)
    for c in range(NCORES):
        b, q0 = c // 2, (c % 2) * TQL
        out[q0:q0 + TQL, b, :] = res.results[c]["out"]
    return out, res


def kernel(query, value, mask, W1, W2, scale):
    out, _ = run(query, value, mask, W1, W2, scale, trace=False)
    return out
